# revision 1
# baseline (speedup 1.0000x reference)
"""Trainium2 Bass kernel for nn_Attention_21878563405851.

Module: kv = x1 @ W_qk (k,v split); q = x2 @ W_v; 8-head attention
(dim_head=64); out @ W_out + b_out.  B=2, N=2048, DIM=512.

Sharding over 8 NeuronCores: core c -> batch b=c//4, heads (2g, 2g+1)
with g=c%4.  Each core:
  1. loads x1[b]^T, x2[b]^T (bf16, host-pretransposed),
  2. computes kT/vT/qT for its 2 heads (d-major) on the PE,
     chunk-pipelined so matmuls start after the first x chunk lands,
  3. flash-style attention in transposed layout, query-pair-major:
     for each 1024-query pair, per 128-key tile dots^T = k @ q^T ->
     exp (ACT, softmax scale folded) -> bf16 e tiles; attn@v
     accumulated as out^T = v_ext^T @ e with a ones column appended to
     v so the softmax denominator falls out of the same matmul
     (row 64 of the accumulator).  Each pair's normalization
     (approx-reciprocal of s + broadcast + DVE multiply) overlaps the
     next pair's matmuls,
  4. per-head AllGather (bf16) of out^T[64, 2048] over the 8 cores;
     the head-0 gather overlaps head-1 compute,
  5. y^T[o_slice] = W_out[:, o_slice]^T @ out^T + b_out[o_slice], one
     64-column slice per core, accumulated in two phases: AG1 chunks
     (overlapping the head-1 AllGather) then AG2 chunks.  The host
     concatenates slices and transposes.  W_out rows are
     host-permuted to the AllGather row order.
"""

import sys

for _p in ("/opt/trn_rl_repo", "/root/.axon_site/_ro/trn_rl_repo"):
    if _p not in sys.path:
        sys.path.insert(0, _p)

import numpy as np
import ml_dtypes

import concourse.bass as bass
import concourse.mybir as mybir
from concourse import tile
from concourse.tile import add_dep_helper
from concourse.bacc import Bacc

B, N, DIM = 2, 2048, 512
HEADS, DH = 8, 64
INNER = HEADS * DH
SCALE = DH ** -0.5
NCORES = 8
HPC = 2            # heads per core
NKT = N // 128     # key tiles
DTW = 1024         # query-pair width (one exp tile)
NQP = N // DTW     # query pairs
NC_CHUNKS = DIM // 128
# AllGather head order: rank r=(b,g) contributes head 2g then 2g+1, so
# gather 0 contains heads [0,2,4,6] per batch, gather 1 heads [1,3,5,7].
HEAD_PERM = [0, 2, 4, 6, 1, 3, 5, 7]

BF16 = mybir.dt.bfloat16
F32 = mybir.dt.float32


def build_program():
    nc = Bacc(None, num_devices=NCORES)

    # ---- external I/O (per core) ----
    xkv = nc.dram_tensor("xkv", [DIM, N], BF16, kind="ExternalInput")   # x1[b]^T
    xq = nc.dram_tensor("xq", [DIM, N], BF16, kind="ExternalInput")     # x2[b]^T
    # weight images host-prearranged in SBUF layout (contiguous DMA):
    # wkv[p, (g c) col], wq[p, (c) col], wo[p, (c) col]
    wkv = nc.dram_tensor("wkv", [128, HPC * NC_CHUNKS * 128], BF16, kind="ExternalInput")
    wq = nc.dram_tensor("wq", [128, NC_CHUNKS * 128], BF16, kind="ExternalInput")
    # W_out[:, o_slice] with rows permuted to HEAD_PERM order
    wo = nc.dram_tensor("wo", [128, NC_CHUNKS * 64], BF16, kind="ExternalInput")
    bo = nc.dram_tensor("bo", [64, 1], F32, kind="ExternalInput")       # b_out[o_slice]
    yT = nc.dram_tensor("yT", [64, B * N], F32, kind="ExternalOutput")

    ident_dram = nc.inline_tensor(
        np.eye(128, dtype=ml_dtypes.bfloat16), name="ident"
    )

    with tile.TileContext(nc) as tc:
        with (
            tc.tile_pool(name="const", bufs=1) as constp,
            tc.tile_pool(name="xin", bufs=1) as xin,
            tc.tile_pool(name="wts", bufs=1) as wts,
            tc.tile_pool(name="kq", bufs=1) as kqp,
            tc.tile_pool(name="vts", bufs=2) as vtsp,
            tc.tile_pool(name="vext", bufs=1) as vextp,
            tc.tile_pool(name="et", bufs=3) as etp,
            tc.tile_pool(name="osb", bufs=1) as osbp,
            tc.tile_pool(name="norm", bufs=2) as normp,
            tc.tile_pool(name="outp", bufs=1) as outp,
            tc.tile_pool(name="gin", bufs=1) as ginp,
            tc.tile_pool(name="yout", bufs=2) as youtp,
            # PSUM: psA = 2 rotating slots of [128,1024] f32 (4 banks) for
            # proj / transpose / dots / final-y tiles; ps_acc = 2 rotating
            # attention accumulator pairs of [65,1024] (4 banks).
            tc.tile_pool(name="psA", bufs=2, space="PSUM") as psA,
            tc.tile_pool(name="ps_acc", bufs=2, space="PSUM") as ps_acc,
            tc.tile_pool(name="dram", bufs=1, space="DRAM") as dramp,
        ):
            # ---- constants ----
            ident = constp.tile([128, 128], BF16, name="ident_s")
            nc.gpsimd.dma_start(ident[:], ident_dram[:])

            # ---- load inputs ----
            # Weight tensors land as single multi-dim-AP DMAs (one queue slot
            # each); xkv chunks go on the SP HWDGE queue, xq on the ACT HWDGE
            # queue so the kv and q input streams run in parallel.
            # weights on the GpSimd (SWDGE) queue so the SP/ACT HWDGE queues
            # carry only the xkv / xq streams.
            wkv_s = wts.tile([128, HPC * NC_CHUNKS * 128], BF16, name="wkv_s")
            nc.gpsimd.dma_start(wkv_s[:], wkv[:])
            xkv_s = xin.tile([128, NC_CHUNKS * N], BF16, name="xkv_s")
            xq_s = xin.tile([128, NC_CHUNKS * N], BF16, name="xq_s")
            for c in range(NC_CHUNKS):
                nc.sync.dma_start(
                    xkv_s[:, c * N:(c + 1) * N], xkv[c * 128:(c + 1) * 128, :]
                )
            wq_s = wts.tile([128, NC_CHUNKS * 128], BF16, name="wq_s")
            nc.gpsimd.dma_start(wq_s[:], wq[:])
            for c in range(NC_CHUNKS):
                nc.scalar.dma_start(
                    xq_s[:, c * N:(c + 1) * N], xq[c * 128:(c + 1) * 128, :]
                )
            wo_s = wts.tile([128, NC_CHUNKS * 64], BF16, name="wo_s")
            nc.gpsimd.dma_start(wo_s[:], wo[:])
            bo_s = wts.tile([64, 1], F32, name="bo_s")
            nc.gpsimd.dma_start(bo_s[:], bo[:])

            kT_s = kqp.tile([128, N], BF16, name="kT_s")
            qT_s = kqp.tile([128, N], BF16, name="qT_s")
            outT_s = outp.tile([128, N], BF16, name="outT_s")

            def projection(w_tile, w_off, x_tile, copy_out):
                """Chunk-outer projection: both [128,1024] psum slots stay
                resident; PE starts after x chunk 0 lands.  copy_out(nt, ps)
                drains psum tile nt (cols nt*DTW..)."""
                pss = [
                    psA.tile([128, DTW], F32, name=f"prj{nt}", tag="ps")
                    for nt in range(N // DTW)
                ]
                for c in range(NC_CHUNKS):
                    for nt in range(N // DTW):
                        for h in range(DTW // 512):
                            col = nt * DTW + h * 512
                            nc.tensor.matmul(
                                pss[nt][:, h * 512:(h + 1) * 512],
                                w_tile[:, w_off + c * 128: w_off + (c + 1) * 128],
                                x_tile[:, c * N + col: c * N + col + 512],
                                start=(c == 0),
                                stop=(c == NC_CHUNKS - 1),
                            )
                for nt in range(N // DTW):
                    copy_out(nt, pss[nt])

            def kv_project(g, vt_s):
                def drain(nt, ps):
                    nc.vector.tensor_copy(
                        kT_s[g * 64:(g + 1) * 64, nt * DTW:(nt + 1) * DTW],
                        ps[0:64, :],
                    )
                    nc.vector.tensor_copy(
                        vt_s[:, nt * DTW:(nt + 1) * DTW], ps[64:128, :]
                    )
                projection(wkv_s, g * NC_CHUNKS * 128, xkv_s, drain)

            def q_project():
                def drain(nt, ps):
                    nc.vector.tensor_copy(
                        qT_s[:, nt * DTW:(nt + 1) * DTW], ps[:]
                    )
                projection(wq_s, 0, xq_s, drain)

            def v_transpose(g, vt_s, v_ext):
                for kt in range(NKT):
                    tr_ps = psA.tile([128, 64], BF16, name="tr_ps", tag="ps")
                    nc.tensor.transpose(
                        tr_ps[:],
                        vt_s[:, kt * 128:(kt + 1) * 128],
                        ident[0:64, 0:64],
                    )
                    nc.vector.tensor_copy(v_ext[:, kt * 66: kt * 66 + 64], tr_ps[:])
                    nc.vector.memset(v_ext[:, kt * 66 + 64: kt * 66 + 65], 1.0)

            # one gather per (head, query-pair): 4 × 128KB; only the last is
            # on the critical tail, the rest overlap compute.
            ag_ins = {}
            ag_outs = {}
            for g in range(HPC):
                for qp in range(NQP):
                    ag_ins[(g, qp)] = dramp.tile(
                        [64, DTW], BF16, name=f"ag_in{g}_{qp}", tag=f"agi{g}{qp}"
                    )
                    ag_outs[(g, qp)] = dramp.tile(
                        [NCORES * 64, DTW], BF16, name=f"ag_out{g}_{qp}",
                        tag=f"ago{g}{qp}", addr_space="Shared",
                    )

            def attention_head(g, v_ext, post_ag=None):
                """Query-pair-major attention + per-pair normalization.

                Software-pipelined emission: the PE queue is in-order, so
                attnv(kt) waiting on exp(kt) would block the already-ready
                dots(kt+1) behind it.  Emitting dots(kt+1) BEFORE attnv(kt)
                lets the PE compute dots while the ACT engine runs exp."""
                accs = [
                    ps_acc.tile([65, DTW], F32, name=f"accq{qp}", tag="acc")
                    for qp in range(NQP)
                ]
                pending = None  # (qp, kt, e_t) awaiting its attnv pair
                last_attnv = [None]

                def emit_attnv(qp, kt, e_t):
                    for h in range(DTW // 512):
                        last_attnv[0] = nc.tensor.matmul(
                            accs[qp][:, h * 512:(h + 1) * 512],
                            v_ext[:, kt * 66: kt * 66 + 65],
                            e_t[:, h * 512:(h + 1) * 512],
                            start=(kt == 0),
                            stop=(kt == NKT - 1),
                        )

                for qp in range(NQP):
                    for kt in range(NKT):
                        dt = psA.tile([128, DTW], F32, name="dt", tag="ps")
                        for h in range(DTW // 512):
                            nc.tensor.matmul(
                                dt[:, h * 512:(h + 1) * 512],
                                kT_s[g * 64:(g + 1) * 64, kt * 128:(kt + 1) * 128],
                                qT_s[g * 64:(g + 1) * 64,
                                     qp * DTW + h * 512: qp * DTW + (h + 1) * 512],
                            )
                        e_t = etp.tile([128, DTW], BF16, name="e_t", tag="e")
                        nc.scalar.activation(
                            e_t[:], dt[:],
                            mybir.ActivationFunctionType.Exp, scale=SCALE,
                        )
                        if pending is not None:
                            emit_attnv(*pending)
                        pending = (qp, kt, e_t)
                emit_attnv(*pending)

                for qp in range(NQP):
                    acc = accs[qp]
                    # drain + normalize this pair; s row copied straight from
                    # PSUM so the reciprocal starts before the big drain.
                    s_s = normp.tile([1, DTW], F32, name="s_s", tag="s1")
                    r_s = normp.tile([1, DTW], F32, name="r_s", tag="s2")
                    rb_s = normp.tile([64, DTW], F32, name="rb_s", tag="rb")
                    nc.vector.tensor_copy(s_s[:], acc[64:65, :])
                    nc.vector.reciprocal_approx_fast(r_s[:], s_s[:])
                    o_sb = osbp.tile([65, DTW], F32, name="o_sb", tag="osb", bufs=2)
                    nc.vector.tensor_copy(o_sb[:], acc[:])
                    r_dram = dramp.tile(
                        [1, DTW], F32, name="r_dram", tag="r_dram", bufs=2
                    )
                    nc.sync.dma_start(r_dram[:], r_s[:])
                    nc.sync.dma_start(
                        rb_s[:], r_dram[0:1, :].broadcast_to([64, DTW])
                    )
                    nc.vector.tensor_mul(
                        outT_s[g * 64:(g + 1) * 64, qp * DTW:(qp + 1) * DTW],
                        o_sb[0:64, :],
                        rb_s[:],
                    )
                    nc.sync.dma_start(
                        ag_ins[(g, qp)][:],
                        outT_s[g * 64:(g + 1) * 64, qp * DTW:(qp + 1) * DTW],
                    )
                    nc.gpsimd.collective_compute(
                        "AllGather",
                        mybir.AluOpType.bypass,
                        replica_groups=[list(range(NCORES))],
                        ins=[ag_ins[(g, qp)][:]],
                        outs=[ag_outs[(g, qp)][:]],
                    )
                return last_attnv[0]

            def load_gts(g, qp, gts, engines):
                """Load the 4 chunks of gather (g, qp) on the given engines.

                Mid-kernel loads go on the otherwise-idle GpSimd (SWDGE)
                queue: a blocked wait for the gather there stalls nothing.
                Putting them on SP/ACT would stall norm DMAs / exps behind
                the AllGather wait."""
                for j, (b, half) in enumerate(
                    [(b, h) for b in range(B) for h in range(2)]
                ):
                    gt = ginp.tile(
                        [128, DTW], BF16,
                        name=f"gt{g}_{qp}_{b}_{half}",
                        tag=f"gt{g}_{qp}_{b}_{half}",
                    )
                    r0 = b * 256 + half * 128
                    eng = engines[j % len(engines)]
                    eng.dma_start(gt[:], ag_outs[(g, qp)][r0: r0 + 128, :])
                    gts[(g, qp, b, half)] = gt

            gts = {}

            # ---- head 0 ----
            vt0 = vtsp.tile([64, N], BF16, name="vt_s")
            kv_project(0, vt0)
            vx0 = vextp.tile([128, NKT * 66], BF16, name="v_ext0", tag="v0")
            v_transpose(0, vt0, vx0)
            q_project()
            attention_head(0, vx0)
            # gather loads AFTER both triggers of the head are queued, so a
            # gt-load's wait never delays a later AG trigger on the queue.
            load_gts(0, 0, gts, [nc.gpsimd])
            load_gts(0, 1, gts, [nc.gpsimd])

            # ---- head 1 (AGs of head 0 run concurrently) ----
            vt1 = vtsp.tile([64, N], BF16, name="vt_s")
            kv_project(1, vt1)
            vx1 = vextp.tile([128, NKT * 66], BF16, name="v_ext1", tag="v1")
            v_transpose(1, vt1, vx1)
            last_attnv1 = attention_head(1, vx1)
            load_gts(1, 0, gts, [nc.gpsimd])
            load_gts(1, 1, gts, [nc.sync, nc.scalar])

            # ---- final projection: y^T[o_slice], two phases ----
            # gather row order per batch b: ag0 -> heads [0,2,4,6],
            # ag1 -> heads [1,3,5,7]; wo rows are HEAD_PERM-permuted.

            # 4 resident y pair-accumulators [64,1024] (8 banks: 2 psA slots
            # + 2 acc slots, all free once head-1 drains).
            y_ps = {}
            for b in range(B):
                for ntp in range(2):
                    pool = psA if (b, ntp) in ((0, 0), (0, 1)) else ps_acc
                    tag = "ps" if pool is psA else "acc"
                    y_ps[(b, ntp)] = pool.tile(
                        [64, DTW], F32, name=f"y{b}{ntp}", tag=tag
                    )
            # phase A: AG1 chunks (heads 0,2,4,6) — overlaps AG2.  The
            # explicit dep keeps the scheduler from interleaving these into
            # the attention stream (their gather-load wait would stall the
            # in-order PE queue).
            # accumulate in gather-arrival order — waves keyed by the gather
            # (ag_g, ntp) they consume, so a late gather never blocks ready
            # work in the in-order PE queue.  The qp=0 output tiles finish
            # at gather (1,0) and drain while gather (1,1) is in flight.
            for ag_g in range(HPC):
                for ntp in range(2):
                    for half in range(2):
                        for b in range(B):
                            for h in range(2):
                                wcol = (ag_g * 2 + half) * 64
                                mm = nc.tensor.matmul(
                                    y_ps[(b, ntp)][:, h * 512:(h + 1) * 512],
                                    wo_s[:, wcol: wcol + 64],
                                    gts[(ag_g, ntp, b, half)][:, h * 512:(h + 1) * 512],
                                    start=(ag_g == 0 and half == 0),
                                    stop=(ag_g == HPC - 1 and half == 1),
                                )
                                if ag_g == 0 and half == 0:
                                    add_dep_helper(
                                        mm.ins,
                                        last_attnv1.ins,
                                        sync=False,
                                        reason="final matmuls after attention",
                                    )
            for ntp in range(2):
                for b in range(B):
                    y_out = youtp.tile([64, DTW], F32, name="y_out", tag="y")
                    nc.scalar.activation(
                        y_out[:], y_ps[(b, ntp)][:],
                        mybir.ActivationFunctionType.Identity, bias=bo_s[:, 0:1],
                    )
                    nc.sync.dma_start(
                        yT[:, b * N + ntp * DTW: b * N + (ntp + 1) * DTW], y_out[:]
                    )

    nc.finalize()
    return nc


_NC_CACHE = None


def _get_program():
    global _NC_CACHE
    if _NC_CACHE is None:
        _NC_CACHE = build_program()
    return _NC_CACHE


def make_in_maps(x1, x2, W_qk, W_v, W_out, b_out):
    bf = ml_dtypes.bfloat16
    x1 = np.asarray(x1, np.float32)
    x2 = np.asarray(x2, np.float32)
    W_qk = np.asarray(W_qk, np.float32)
    W_v = np.asarray(W_v, np.float32)
    W_out = np.asarray(W_out, np.float32)
    b_out = np.asarray(b_out, np.float32)

    x1T = [np.ascontiguousarray(x1[b].T).astype(bf) for b in range(B)]
    x2T = [np.ascontiguousarray(x2[b].T).astype(bf) for b in range(B)]
    # W_out rows in AllGather order
    W_out_perm = np.concatenate(
        [W_out[h * DH:(h + 1) * DH, :] for h in HEAD_PERM], axis=0
    )

    in_maps = []
    for c in range(NCORES):
        b = c // 4
        g = c % 4
        heads = (2 * g, 2 * g + 1)
        wkv_arr = np.stack(
            [
                np.concatenate(
                    [
                        W_qk[:, h * DH:(h + 1) * DH],
                        W_qk[:, INNER + h * DH: INNER + (h + 1) * DH],
                    ],
                    axis=1,
                )
                for h in heads
            ]
        ).astype(bf)
        wq_arr = np.concatenate(
            [W_v[:, h * DH:(h + 1) * DH] for h in heads], axis=1
        ).astype(bf)
        osl = slice(c * 64, (c + 1) * 64)
        # pre-arrange weight images in SBUF layout: partition p holds row
        # (chunk*128 + p) of the [DIM, cols] weight, chunks along free dim.
        wkv_img = (
            wkv_arr.reshape(HPC, NC_CHUNKS, 128, 128)
            .transpose(2, 0, 1, 3)
            .reshape(128, HPC * NC_CHUNKS * 128)
        )
        wq_img = (
            wq_arr.reshape(NC_CHUNKS, 128, 128)
            .transpose(1, 0, 2)
            .reshape(128, NC_CHUNKS * 128)
        )
        wo_img = (
            W_out_perm[:, osl].astype(bf)
            .reshape(NC_CHUNKS, 128, 64)
            .transpose(1, 0, 2)
            .reshape(128, NC_CHUNKS * 64)
        )
        in_maps.append(
            {
                "xkv": x1T[b],
                "xq": x2T[b],
                "wkv": np.ascontiguousarray(wkv_img),
                "wq": np.ascontiguousarray(wq_img),
                "wo": np.ascontiguousarray(wo_img),
                "bo": np.ascontiguousarray(b_out[osl].reshape(64, 1)),
            }
        )
    return in_maps


def assemble_output(results):
    y = np.concatenate([results[c]["yT"] for c in range(NCORES)], axis=0)
    return np.ascontiguousarray(y.T.reshape(B, N, DIM)).astype(np.float32)


def kernel(x1, x2, W_qk, W_v, W_out, b_out):
    from concourse.bass_utils import run_bass_kernel_spmd

    nc = _get_program()
    in_maps = make_in_maps(x1, x2, W_qk, W_v, W_out, b_out)
    res = run_bass_kernel_spmd(nc, in_maps, list(range(NCORES)))
    return assemble_output(res.results)



# revision 7
# speedup vs baseline: 1.3482x; 1.3482x over previous
"""Trainium2 Bass kernel for nn_Attention_21878563405851.

Module: kv = x1 @ W_qk (k,v split); q = x2 @ W_v; 8-head attention
(dim_head=64); out @ W_out + b_out.  B=2, N=2048, DIM=512.

Sharding over 8 NeuronCores: core c -> batch b=c//4, query chunk
qc=c%4 (512 queries), ALL 8 heads.  Fully collective-free: the kv
projection is recomputed on each of the 4 cores of a batch group
(cheaper than this fabric's AllGather), and each core's output slice
y[b, qc*512:(qc+1)*512, :] is disjoint.

Per core:
  1. q proj (x2 slice, d-major), v proj (x1 full, key-major with a
     ones column appended per head so the softmax denominator falls
     out of the attnv matmul), k proj (x1 full, d-major).
  2. per head h: dots^T[kt] = k_h @ q_h^T -> exp (ACT, scale folded)
     -> attnv accumulated into [65, 512] PSUM (row 64 = denominator).
     Software-pipelined: dots(kt+1) is emitted before attnv(kt) so the
     in-order PE queue never waits on the ACT exp.
  3. normalization: reciprocal of row 64 (DVE), partition-broadcast
     (GpSimd) to 64 rows, DVE multiply into attn-out (bf16).
  4. out proj per head-pair into PSUM, accumulated in SBUF f32 (bias
     folded into the first accumulation), y^T DMA'd out per dim-group.
"""

import sys

for _p in ("/opt/trn_rl_repo", "/root/.axon_site/_ro/trn_rl_repo"):
    if _p not in sys.path:
        sys.path.insert(0, _p)

import numpy as np
import ml_dtypes

import concourse.bass as bass
import concourse.mybir as mybir
from concourse import tile
from concourse.bacc import Bacc

B, N, DIM = 2, 2048, 512
HEADS, DH = 8, 64
INNER = HEADS * DH
SCALE = DH ** -0.5
NCORES = 8
NQ = 512           # queries per core
NKT = N // 128     # 16 key tiles
NC = DIM // 128    # 4 contraction chunks

BF16 = mybir.dt.bfloat16
F32 = mybir.dt.float32


def build_program():
    nc = Bacc(None, num_devices=NCORES)

    # ---- external I/O (per core), host-prearranged SBUF images ----
    x1T = nc.dram_tensor("x1T", [128, NC * N], BF16, kind="ExternalInput")
    x2T = nc.dram_tensor("x2T", [128, NC * NQ], BF16, kind="ExternalInput")
    wk = nc.dram_tensor("wk", [128, 4 * NC * 128], BF16, kind="ExternalInput")
    wq = nc.dram_tensor("wq", [128, 4 * NC * 128], BF16, kind="ExternalInput")
    wv = nc.dram_tensor("wv", [128, NC * INNER], BF16, kind="ExternalInput")
    wo = nc.dram_tensor("wo", [128, 4 * 4 * 128], BF16, kind="ExternalInput")
    bo = nc.dram_tensor("bo", [128, 4], F32, kind="ExternalInput")
    yT = nc.dram_tensor("yT", [128, 4 * NQ], F32, kind="ExternalOutput")

    with tile.TileContext(nc) as tc:
        with (
            tc.tile_pool(name="xin", bufs=1) as xin,
            tc.tile_pool(name="wts", bufs=1) as wts,
            tc.tile_pool(name="kq", bufs=1) as kqp,
            tc.tile_pool(name="vex", bufs=1) as vexp,
            tc.tile_pool(name="et", bufs=3) as etp,
            tc.tile_pool(name="os", bufs=1) as osp,
            tc.tile_pool(name="ysb", bufs=1) as ysbp,
            tc.tile_pool(name="nrm", bufs=2) as nrmp,
            tc.tile_pool(name="dram", bufs=1, space="DRAM") as dramp,
            # PSUM (8 banks): tag "big" [128,1024] x2 (4 banks) for k-proj
            # halves and dots pairs; tag "s5" [128,512] x2 (2 banks) for
            # q/v proj and out-proj partials; tag "acc" [128,512] x2
            # (2 banks) for the long-lived attnv accumulators.
            tc.tile_pool(name="ps", bufs=1, space="PSUM") as psp,
        ):
            # ---- load inputs ----
            x2T_s = xin.tile([128, NC * NQ], BF16, name="x2T_s")
            nc.scalar.dma_start(x2T_s[:], x2T[:])
            wq_s = wts.tile([128, 4 * NC * 128], BF16, name="wq_s")
            nc.gpsimd.dma_start(wq_s[:], wq[:])
            x1T_s = xin.tile([128, NC * N], BF16, name="x1T_s")
            for c in range(NC):
                nc.sync.dma_start(
                    x1T_s[:, c * N:(c + 1) * N], x1T[:, c * N:(c + 1) * N]
                )
            wv_s = wts.tile([128, NC * INNER], BF16, name="wv_s")
            nc.gpsimd.dma_start(wv_s[:], wv[:])
            wk_s = wts.tile([128, 4 * NC * 128], BF16, name="wk_s")
            nc.gpsimd.dma_start(wk_s[:], wk[:])
            wo_s = wts.tile([128, 4 * 4 * 128], BF16, name="wo_s")
            nc.gpsimd.dma_start(wo_s[:], wo[:])
            bo_s = wts.tile([128, 4], F32, name="bo_s")
            nc.gpsimd.dma_start(bo_s[:], bo[:])

            qT_s = kqp.tile([128, 4 * NQ], BF16, name="qT_s")
            kT_s = kqp.tile([128, 4 * N], BF16, name="kT_s")
            # v extended: per key tile, per head: 64 v cols + 1 ones col
            vE_s = vexp.tile([128, NKT, HEADS, 65], BF16, name="vE_s")
            nc.vector.memset(vE_s[:, :, :, 64:65], 1.0)

            o_s = osp.tile([128, 4, NQ], BF16, name="o_s")
            y_sb = ysbp.tile([128, 4, NQ], F32, name="y_sb")

            # ---- q projection: qT[128 (2h,d), 512] per head-pair group ----
            for g in range(4):
                ps = psp.tile([128, NQ], F32, name="psq", tag="s5", bufs=2)
                for c in range(NC):
                    nc.tensor.matmul(
                        ps[:],
                        wq_s[:, (g * NC + c) * 128:(g * NC + c + 1) * 128],
                        x2T_s[:, c * NQ:(c + 1) * NQ],
                        start=(c == 0),
                        stop=(c == NC - 1),
                    )
                nc.vector.tensor_copy(qT_s[:, g * NQ:(g + 1) * NQ], ps[:])

            # ---- v projection (key-major): [128 keys, 512 (h,d)] per kt ----
            for kt in range(NKT):
                ps = psp.tile([128, INNER], F32, name="psv", tag="s5", bufs=2)
                for c in range(NC):
                    nc.tensor.matmul(
                        ps[:],
                        x1T_s[:, c * N + kt * 128: c * N + (kt + 1) * 128],
                        wv_s[:, c * INNER:(c + 1) * INNER],
                        start=(c == 0),
                        stop=(c == NC - 1),
                    )
                nc.vector.tensor_copy(
                    vE_s[:, kt, :, 0:64],
                    ps.rearrange("p (h d) -> p h d", h=HEADS),
                )

            # ---- k projection (d-major): [128 (2h,d), 2048] per group ----
            for g in range(4):
                halves = [
                    psp.tile([128, 1024], F32, name=f"psk{g}{i}", tag="big", bufs=2)
                    for i in range(2)
                ]
                for c in range(NC):
                    for half in range(2):
                        for j in range(2):
                            col = half * 1024 + j * 512
                            nc.tensor.matmul(
                                halves[half][:, j * 512:(j + 1) * 512],
                                wk_s[:, (g * NC + c) * 128:(g * NC + c + 1) * 128],
                                x1T_s[:, c * N + col: c * N + col + 512],
                                start=(c == 0),
                                stop=(c == NC - 1),
                            )
                for half in range(2):
                    nc.vector.tensor_copy(
                        kT_s[:, g * N + half * 1024: g * N + (half + 1) * 1024],
                        halves[half][:],
                    )

            # ---- attention; cross-engine emits deferred so the in-order
            # PE queue never waits on DVE/ACT results ----
            deferred = []
            for h in range(HEADS):
                g, hl = h // 2, h % 2
                r0 = hl * 64
                acc = psp.tile([128, NQ], F32, name=f"acc{h}", tag="acc", bufs=2)
                pending = None

                def emit_attnv(kp, e_t, acc=acc, h=h):
                    for j in range(2):
                        kt = 2 * kp + j
                        nc.tensor.matmul(
                            acc[0:65, :],
                            vE_s[:, kt, h, :],
                            e_t[:, j * 512:(j + 1) * 512],
                            start=(kt == 0),
                            stop=(kt == NKT - 1),
                        )

                for kp in range(NKT // 2):
                    dt = psp.tile([128, 1024], F32, name="dt", tag="big", bufs=2)
                    for j in range(2):
                        kt = 2 * kp + j
                        nc.tensor.matmul(
                            dt[:, j * 512:(j + 1) * 512],
                            kT_s[r0:r0 + 64, g * N + kt * 128: g * N + (kt + 1) * 128],
                            qT_s[r0:r0 + 64, g * NQ:(g + 1) * NQ],
                        )
                    e_t = etp.tile([128, 1024], BF16, name="e_t", tag="e")
                    nc.scalar.activation(
                        e_t[:], dt[:],
                        mybir.ActivationFunctionType.Exp, scale=SCALE,
                    )
                    if kp == 1 and deferred:
                        for fn in deferred:
                            fn()
                        deferred = []
                    if pending is not None:
                        emit_attnv(*pending)
                    pending = (kp, e_t)
                emit_attnv(*pending)

                # normalization: reciprocal now (DVE); broadcast (GpSimd) +
                # multiply (DVE) + out-proj (PE) deferred into next head
                # reciprocal_approx_fast (custom DVE ucode) cannot read PSUM
                # on hw — copy the denominator row to SBUF first.
                s_s = nrmp.tile([1, NQ], F32, name="s_s", tag="s")
                nc.vector.tensor_copy(s_s[:], acc[64:65, :])
                r_s = nrmp.tile([1, NQ], F32, name="r_s", tag="r")
                nc.vector.reciprocal_approx_fast(r_s[:], s_s[:])
                # broadcast across partitions via a DRAM round trip (the
                # proven-on-hw pattern); runs on the idle SP DMA queue
                r_dram = dramp.tile([1, NQ], F32, name="r_dram", tag="rd", bufs=2)
                nc.sync.dma_start(r_dram[:], r_s[:])
                rb_s = nrmp.tile([64, NQ], F32, name="rb_s", tag="rb")
                nc.sync.dma_start(rb_s[:], r_dram[0:1, :].broadcast_to([64, NQ]))

                def emit_mult(acc=acc, rb_s=rb_s, g=g, hl=hl):
                    nc.vector.tensor_mul(
                        o_s[hl * 64:(hl + 1) * 64, g, :], acc[0:64, :], rb_s[:]
                    )
                deferred.append(emit_mult)

                if hl == 1:
                    def emit_y(p=g):
                        for dg in range(4):
                            yp = psp.tile(
                                [128, NQ], F32, name=f"yp{p}{dg}", tag="s5", bufs=2
                            )
                            nc.tensor.matmul(
                                yp[:],
                                wo_s[:, (dg * 4 + p) * 128:(dg * 4 + p + 1) * 128],
                                o_s[:, p, :],
                            )
                            if p == 0:
                                nc.vector.tensor_scalar_add(
                                    y_sb[:, dg, :], yp[:], bo_s[:, dg:dg + 1]
                                )
                            else:
                                nc.vector.tensor_tensor(
                                    y_sb[:, dg, :], y_sb[:, dg, :], yp[:],
                                    mybir.AluOpType.add,
                                )
                    deferred.append(emit_y)

            # flush remaining deferred work (last head's norm + out-proj)
            for fn in deferred:
                fn()

            # ---- final output DMA ----
            for dg in range(4):
                nc.sync.dma_start(
                    yT[:, dg * NQ:(dg + 1) * NQ], y_sb[:, dg, :]
                )

    nc.finalize()
    return nc


_NC_CACHE = None


def _get_program():
    global _NC_CACHE
    if _NC_CACHE is None:
        _NC_CACHE = build_program()
    return _NC_CACHE


def make_in_maps(x1, x2, W_qk, W_v, W_out, b_out):
    bf = ml_dtypes.bfloat16
    x1 = np.asarray(x1, np.float32)
    x2 = np.asarray(x2, np.float32)
    W_qk = np.asarray(W_qk, np.float32)
    W_v = np.asarray(W_v, np.float32)
    W_out = np.asarray(W_out, np.float32)
    b_out = np.asarray(b_out, np.float32)

    # weight images, shared by all cores
    # wk/wq: [p, (g c) f] = W[c*128+p, g*128+f]
    def stat_img(W):
        return np.ascontiguousarray(
            W.reshape(NC, 128, 4, 128).transpose(1, 2, 0, 3).reshape(128, 2048)
        ).astype(bf)

    wk_img = stat_img(W_qk[:, :INNER])
    wq_img = stat_img(W_v)
    # wv: [p, c f] = W_qk[c*128+p, 512+f]
    wv_img = np.ascontiguousarray(
        W_qk[:, INNER:].reshape(NC, 128, INNER).transpose(1, 0, 2).reshape(128, NC * INNER)
    ).astype(bf)
    # wo: [p, (dg pp) f] = W_out[pp*128+p, dg*128+f]
    wo_img = np.ascontiguousarray(
        W_out.reshape(4, 128, 4, 128).transpose(1, 2, 0, 3).reshape(128, 2048)
    ).astype(bf)
    bo_img = np.ascontiguousarray(b_out.reshape(4, 128).T)

    x1T_imgs = [
        np.ascontiguousarray(
            x1[b].reshape(N, NC, 128).transpose(2, 1, 0).reshape(128, NC * N)
        ).astype(bf)
        for b in range(B)
    ]

    in_maps = []
    for c in range(NCORES):
        b, qc = c // 4, c % 4
        qs = qc * NQ
        x2T_img = np.ascontiguousarray(
            x2[b, qs:qs + NQ].reshape(NQ, NC, 128).transpose(2, 1, 0).reshape(128, NC * NQ)
        ).astype(bf)
        in_maps.append(
            {
                "x1T": x1T_imgs[b],
                "x2T": x2T_img,
                "wk": wk_img,
                "wq": wq_img,
                "wv": wv_img,
                "wo": wo_img,
                "bo": bo_img,
            }
        )
    return in_maps


def assemble_output(results):
    y = np.empty((B, N, DIM), np.float32)
    for c in range(NCORES):
        b, qc = c // 4, c % 4
        yTc = np.asarray(results[c]["yT"])  # [128, 4*512]
        D = yTc.reshape(128, 4, NQ).transpose(1, 0, 2).reshape(DIM, NQ)
        y[b, qc * NQ:(qc + 1) * NQ, :] = D.T
    return y


def kernel(x1, x2, W_qk, W_v, W_out, b_out):
    from concourse.bass_utils import run_bass_kernel_spmd

    nc = _get_program()
    in_maps = make_in_maps(x1, x2, W_qk, W_v, W_out, b_out)
    res = run_bass_kernel_spmd(nc, in_maps, list(range(NCORES)))
    return assemble_output(res.results)


# revision 11
# speedup vs baseline: 1.3696x; 1.0159x over previous
"""Trainium2 Bass kernel for nn_Attention_21878563405851.

Module: kv = x1 @ W_qk (k,v split); q = x2 @ W_v; 8-head attention
(dim_head=64); out @ W_out + b_out.  B=2, N=2048, DIM=512.

Sharding over 8 NeuronCores: core c -> batch b=c//4, query chunk
qc=c%4 (512 queries), ALL 8 heads.  Fully collective-free: the kv
projection is recomputed on each of the 4 cores of a batch group
(cheaper than this fabric's AllGather), and each core's output slice
y[b, qc*512:(qc+1)*512, :] is disjoint.

Per core:
  1. q proj (x2 slice, d-major), v proj (x1 full, key-major with a
     ones column appended per head so the softmax denominator falls
     out of the attnv matmul), k proj (x1 full, d-major).
  2. per head h: dots^T[kt] = k_h @ q_h^T -> exp (ACT, scale folded)
     -> attnv accumulated into [65, 512] PSUM (row 64 = denominator).
     Software-pipelined: dots(kt+1) is emitted before attnv(kt) so the
     in-order PE queue never waits on the ACT exp.
  3. normalization: reciprocal of row 64 (DVE), partition-broadcast
     (GpSimd) to 64 rows, DVE multiply into attn-out (bf16).
  4. out proj per head-pair into PSUM, accumulated in SBUF f32 (bias
     folded into the first accumulation), y^T DMA'd out per dim-group.
"""

import sys

for _p in ("/opt/trn_rl_repo", "/root/.axon_site/_ro/trn_rl_repo"):
    if _p not in sys.path:
        sys.path.insert(0, _p)

import numpy as np
import ml_dtypes

import concourse.bass as bass
import concourse.mybir as mybir
from concourse import tile
from concourse.bacc import Bacc

B, N, DIM = 2, 2048, 512
HEADS, DH = 8, 64
INNER = HEADS * DH
SCALE = DH ** -0.5
NCORES = 8
NQ = 512           # queries per core
NKT = N // 128     # 16 key tiles
NC = DIM // 128    # 4 contraction chunks

BF16 = mybir.dt.bfloat16
F32 = mybir.dt.float32


def build_program():
    nc = Bacc(None, num_devices=NCORES)

    # ---- external I/O (per core), host-prearranged SBUF images ----
    x1T = nc.dram_tensor("x1T", [128, NC * N], BF16, kind="ExternalInput")
    x2T = nc.dram_tensor("x2T", [128, NC * NQ], BF16, kind="ExternalInput")
    wk = nc.dram_tensor("wk", [128, 4 * NC * 128], BF16, kind="ExternalInput")
    wq = nc.dram_tensor("wq", [128, 4 * NC * 128], BF16, kind="ExternalInput")
    wv = nc.dram_tensor("wv", [128, NC * INNER], BF16, kind="ExternalInput")
    wo = nc.dram_tensor("wo", [128, 4 * 4 * 128], BF16, kind="ExternalInput")
    bo = nc.dram_tensor("bo", [128, 4], F32, kind="ExternalInput")
    yT = nc.dram_tensor("yT", [128, 4 * NQ], F32, kind="ExternalOutput")

    with tile.TileContext(nc) as tc:
        with (
            tc.tile_pool(name="xin", bufs=1) as xin,
            tc.tile_pool(name="wts", bufs=1) as wts,
            tc.tile_pool(name="kq", bufs=1) as kqp,
            tc.tile_pool(name="vex", bufs=1) as vexp,
            tc.tile_pool(name="et", bufs=3) as etp,
            tc.tile_pool(name="os", bufs=1) as osp,
            tc.tile_pool(name="ysb", bufs=1) as ysbp,
            tc.tile_pool(name="nrm", bufs=2) as nrmp,
            tc.tile_pool(name="dram", bufs=1, space="DRAM") as dramp,
            # PSUM (8 banks): tag "big" [128,1024] x2 (4 banks) for k-proj
            # halves and dots pairs; tag "s5" [128,512] x2 (2 banks) for
            # q/v proj and out-proj partials; tag "acc" [128,512] x2
            # (2 banks) for the long-lived attnv accumulators.
            tc.tile_pool(name="ps", bufs=1, space="PSUM") as psp,
        ):
            # ---- load inputs ----
            x2T_s = xin.tile([128, NC * NQ], BF16, name="x2T_s")
            nc.scalar.dma_start(x2T_s[:], x2T[:])
            wq_s = wts.tile([128, 4 * NC * 128], BF16, name="wq_s")
            nc.sync.dma_start(wq_s[:], wq[:])
            wk_s = wts.tile([128, 4 * NC * 128], BF16, name="wk_s")
            nc.gpsimd.dma_start(wk_s[:], wk[:])
            x1T_s = xin.tile([128, NC * N], BF16, name="x1T_s")
            for c in range(NC):
                eng = nc.sync if c < 2 else nc.scalar
                eng.dma_start(
                    x1T_s[:, c * N:(c + 1) * N], x1T[:, c * N:(c + 1) * N]
                )
            wv_s = wts.tile([128, NC * INNER], BF16, name="wv_s")
            nc.gpsimd.dma_start(wv_s[:], wv[:])
            wo_s = wts.tile([128, 4 * 4 * 128], BF16, name="wo_s")
            nc.gpsimd.dma_start(wo_s[:], wo[:])
            bo_s = wts.tile([128, 4], F32, name="bo_s")
            nc.gpsimd.dma_start(bo_s[:], bo[:])

            qT_s = kqp.tile([128, 4 * NQ], BF16, name="qT_s")
            kT_s = kqp.tile([128, 4 * N], BF16, name="kT_s")
            # v extended: per key tile, per head: 64 v cols + 1 ones col
            vE_s = vexp.tile([128, NKT, HEADS, 65], BF16, name="vE_s")
            nc.vector.memset(vE_s[:, :, :, 64:65], 1.0)

            o_s = osp.tile([128, 4, NQ], BF16, name="o_s")
            y_sb = ysbp.tile([128, 4, NQ], F32, name="y_sb")

            # ---- q projection: qT[128 (2h,d), 512] per head-pair group ----
            for g in range(4):
                ps = psp.tile([128, NQ], F32, name="psq", tag="s5", bufs=2)
                for c in range(NC):
                    nc.tensor.matmul(
                        ps[:],
                        wq_s[:, (g * NC + c) * 128:(g * NC + c + 1) * 128],
                        x2T_s[:, c * NQ:(c + 1) * NQ],
                        start=(c == 0),
                        stop=(c == NC - 1),
                    )
                nc.vector.tensor_copy(qT_s[:, g * NQ:(g + 1) * NQ], ps[:])

            # ---- k projection (d-major): [128 (2h,d), 2048] per group ----
            for g in range(4):
                halves = [
                    psp.tile([128, 1024], F32, name=f"psk{g}{i}", tag="big", bufs=2)
                    for i in range(2)
                ]
                for c in range(NC):
                    for half in range(2):
                        for j in range(2):
                            col = half * 1024 + j * 512
                            nc.tensor.matmul(
                                halves[half][:, j * 512:(j + 1) * 512],
                                wk_s[:, (g * NC + c) * 128:(g * NC + c + 1) * 128],
                                x1T_s[:, c * N + col: c * N + col + 512],
                                start=(c == 0),
                                stop=(c == NC - 1),
                            )
                        if c == NC - 1:
                            # drain each half as soon as it completes so the
                            # next group's matmuls aren't blocked on both
                            nc.vector.tensor_copy(
                                kT_s[:, g * N + half * 1024:
                                     g * N + (half + 1) * 1024],
                                halves[half][:],
                            )

            # ---- v projection (key-major): [128 keys, 512 (h,d)] per kt ----
            for kt in range(NKT):
                ps = psp.tile([128, INNER], F32, name="psv", tag="s5", bufs=2)
                for c in range(NC):
                    nc.tensor.matmul(
                        ps[:],
                        x1T_s[:, c * N + kt * 128: c * N + (kt + 1) * 128],
                        wv_s[:, c * INNER:(c + 1) * INNER],
                        start=(c == 0),
                        stop=(c == NC - 1),
                    )
                nc.vector.tensor_copy(
                    vE_s[:, kt, :, 0:64],
                    ps.rearrange("p (h d) -> p h d", h=HEADS),
                )

            # ---- attention; cross-engine emits deferred so the in-order
            # PE queue never waits on DVE/ACT results ----
            deferred = []
            for h in range(HEADS):
                g, hl = h // 2, h % 2
                r0 = hl * 64
                acc = psp.tile([128, NQ], F32, name=f"acc{h}", tag="acc", bufs=2)
                pending = None

                def emit_attnv(kp, e_t, acc=acc, h=h):
                    for j in range(2):
                        kt = 2 * kp + j
                        nc.tensor.matmul(
                            acc[0:65, :],
                            vE_s[:, kt, h, :],
                            e_t[:, j * 512:(j + 1) * 512],
                            start=(kt == 0),
                            stop=(kt == NKT - 1),
                        )

                for kp in range(NKT // 2):
                    dt = psp.tile([128, 1024], F32, name="dt", tag="big", bufs=2)
                    for j in range(2):
                        kt = 2 * kp + j
                        nc.tensor.matmul(
                            dt[:, j * 512:(j + 1) * 512],
                            kT_s[r0:r0 + 64, g * N + kt * 128: g * N + (kt + 1) * 128],
                            qT_s[r0:r0 + 64, g * NQ:(g + 1) * NQ],
                        )
                    e_t = etp.tile([128, 1024], BF16, name="e_t", tag="e")
                    nc.scalar.activation(
                        e_t[:], dt[:],
                        mybir.ActivationFunctionType.Exp, scale=SCALE,
                    )
                    if kp == 1 and deferred:
                        for fn in deferred:
                            fn()
                        deferred = []
                    if pending is not None:
                        emit_attnv(*pending)
                    pending = (kp, e_t)
                emit_attnv(*pending)

                # normalization: reciprocal now (DVE); broadcast (GpSimd) +
                # multiply (DVE) + out-proj (PE) deferred into next head
                # reciprocal_approx_fast (custom DVE ucode) cannot read PSUM
                # on hw — copy the denominator row to SBUF first.
                s_s = nrmp.tile([1, NQ], F32, name="s_s", tag="s")
                nc.vector.tensor_copy(s_s[:], acc[64:65, :])
                r_s = nrmp.tile([1, NQ], F32, name="r_s", tag="r")
                nc.vector.reciprocal_approx_fast(r_s[:], s_s[:])
                # broadcast across partitions on the (idle) GpSimd engine
                rb_s = nrmp.tile([64, NQ], F32, name="rb_s", tag="rb")
                nc.gpsimd.partition_broadcast(rb_s[:], r_s[:])

                def emit_mult(acc=acc, rb_s=rb_s, g=g, hl=hl):
                    nc.vector.tensor_mul(
                        o_s[hl * 64:(hl + 1) * 64, g, :], acc[0:64, :], rb_s[:]
                    )
                deferred.append(emit_mult)

                if hl == 1:
                    def emit_y(p=g):
                        for dg in range(4):
                            yp = psp.tile(
                                [128, NQ], F32, name=f"yp{p}{dg}", tag="s5", bufs=2
                            )
                            nc.tensor.matmul(
                                yp[:],
                                wo_s[:, (dg * 4 + p) * 128:(dg * 4 + p + 1) * 128],
                                o_s[:, p, :],
                            )
                            if p == 0:
                                nc.vector.tensor_scalar_add(
                                    y_sb[:, dg, :], yp[:], bo_s[:, dg:dg + 1]
                                )
                            else:
                                nc.vector.tensor_tensor(
                                    y_sb[:, dg, :], y_sb[:, dg, :], yp[:],
                                    mybir.AluOpType.add,
                                )
                    deferred.append(emit_y)

            # flush remaining deferred work (last head's norm + out-proj)
            for fn in deferred:
                fn()

            # ---- final output DMA (spread across queues) ----
            for dg, eng in enumerate(
                (nc.sync, nc.scalar, nc.gpsimd, nc.sync)
            ):
                eng.dma_start(yT[:, dg * NQ:(dg + 1) * NQ], y_sb[:, dg, :])

    nc.finalize()
    return nc


_NC_CACHE = None


def _get_program():
    global _NC_CACHE
    if _NC_CACHE is None:
        _NC_CACHE = build_program()
    return _NC_CACHE


def make_in_maps(x1, x2, W_qk, W_v, W_out, b_out):
    bf = ml_dtypes.bfloat16
    x1 = np.asarray(x1, np.float32)
    x2 = np.asarray(x2, np.float32)
    W_qk = np.asarray(W_qk, np.float32)
    W_v = np.asarray(W_v, np.float32)
    W_out = np.asarray(W_out, np.float32)
    b_out = np.asarray(b_out, np.float32)

    # weight images, shared by all cores
    # wk/wq: [p, (g c) f] = W[c*128+p, g*128+f]
    def stat_img(W):
        return np.ascontiguousarray(
            W.reshape(NC, 128, 4, 128).transpose(1, 2, 0, 3).reshape(128, 2048)
        ).astype(bf)

    wk_img = stat_img(W_qk[:, :INNER])
    wq_img = stat_img(W_v)
    # wv: [p, c f] = W_qk[c*128+p, 512+f]
    wv_img = np.ascontiguousarray(
        W_qk[:, INNER:].reshape(NC, 128, INNER).transpose(1, 0, 2).reshape(128, NC * INNER)
    ).astype(bf)
    # wo: [p, (dg pp) f] = W_out[pp*128+p, dg*128+f]
    wo_img = np.ascontiguousarray(
        W_out.reshape(4, 128, 4, 128).transpose(1, 2, 0, 3).reshape(128, 2048)
    ).astype(bf)
    bo_img = np.ascontiguousarray(b_out.reshape(4, 128).T)

    x1T_imgs = [
        np.ascontiguousarray(
            x1[b].reshape(N, NC, 128).transpose(2, 1, 0).reshape(128, NC * N)
        ).astype(bf)
        for b in range(B)
    ]

    in_maps = []
    for c in range(NCORES):
        b, qc = c // 4, c % 4
        qs = qc * NQ
        x2T_img = np.ascontiguousarray(
            x2[b, qs:qs + NQ].reshape(NQ, NC, 128).transpose(2, 1, 0).reshape(128, NC * NQ)
        ).astype(bf)
        in_maps.append(
            {
                "x1T": x1T_imgs[b],
                "x2T": x2T_img,
                "wk": wk_img,
                "wq": wq_img,
                "wv": wv_img,
                "wo": wo_img,
                "bo": bo_img,
            }
        )
    return in_maps


def assemble_output(results):
    y = np.empty((B, N, DIM), np.float32)
    for c in range(NCORES):
        b, qc = c // 4, c % 4
        yTc = np.asarray(results[c]["yT"])  # [128, 4*512]
        D = yTc.reshape(128, 4, NQ).transpose(1, 0, 2).reshape(DIM, NQ)
        y[b, qc * NQ:(qc + 1) * NQ, :] = D.T
    return y


def kernel(x1, x2, W_qk, W_v, W_out, b_out):
    from concourse.bass_utils import run_bass_kernel_spmd

    nc = _get_program()
    in_maps = make_in_maps(x1, x2, W_qk, W_v, W_out, b_out)
    res = run_bass_kernel_spmd(nc, in_maps, list(range(NCORES)))
    return assemble_output(res.results)
